# revision 4
# baseline (speedup 1.0000x reference)
"""Trainium2 Bass kernel for masked multi-adaptor LoRA:

    y = x @ W^T + b + sum_n mask[n] * SCALE * ((x @ A[n]^T) @ Bw[n]^T)

Strategy (8 NeuronCores, data-parallel over tokens):
  - Flatten x to [B*S, D] = [16384, 2048] tokens; each core takes 2048 tokens.
  - Host pre-transposes/casts: xT [D, T] bf16 per core, WT = W^T [D, O] bf16,
    AT = A_cat^T [D, 64] bf16, BwT = Bw_cat [(n,r), O] bf16,
    m64[(n,r), t] = mask[n, t] * SCALE f32 (mask folded before 2nd LoRA matmul).
  - Device per core:  hT[(n,r),t] = AT.T @ xT   (K=D, 16 k-tiles)
                      gT = hT * m64             (DVE, cast to bf16)
                      psum[t,o] = sum_k xT_k.T @ WT_k  +  gT.T @ BwT  (K=64 tail)
  - Everything (xT, WT) stays resident in SBUF; single pass, PE-bound.
  - b is added on host (zeros in this problem, but kept for generality).
"""

import os
import sys

if "/opt/trn_rl_repo" not in sys.path:
    sys.path.insert(0, "/opt/trn_rl_repo")

import numpy as np
import ml_dtypes

import concourse.mybir as mybir
import concourse.tile as tile
from concourse import bacc
from concourse.bass_utils import run_bass_kernel_spmd

N_CORES = 8
D = 2048          # d_in
O = 2048          # d_out
T = 2048          # tokens per core (16384 / 8)
NR = 64           # n_adaptors * r = 4 * 16
KT = D // 128     # 16 k-tiles
NCH = T // 512    # 512-token chunks for the h phase
NO = O // 512     # 512-wide output column tiles
NTS = T // 128    # 128-token output row tiles
SCALE = 2.0       # lora_alpha / r = 32 / 16

BF16 = mybir.dt.bfloat16
F32 = mybir.dt.float32

_NC = None


def _build():
    nc = bacc.Bacc("TRN2", target_bir_lowering=False, debug=False)
    xT = nc.dram_tensor("xT", [D, T], BF16, kind="ExternalInput").ap()
    wT = nc.dram_tensor("wT", [D, O], BF16, kind="ExternalInput").ap()
    aT = nc.dram_tensor("aT", [D, NR], BF16, kind="ExternalInput").ap()
    bwT = nc.dram_tensor("bwT", [NR, O], BF16, kind="ExternalInput").ap()
    m64 = nc.dram_tensor("m64", [NR, T], F32, kind="ExternalInput").ap()
    y = nc.dram_tensor("y", [T, O], F32, kind="ExternalOutput").ap()

    with tile.TileContext(nc) as tc:
        with (
            tc.tile_pool(name="big", bufs=1) as big,
            tc.tile_pool(name="outp", bufs=6) as outp,
            tc.tile_pool(name="psum", bufs=8, space="PSUM") as psum,
        ):
            # ---- resident loads ----
            m64_sb = big.tile([NR, T], F32, tag="m64_sb")
            nc.sync.dma_start(m64_sb, m64)
            bwT_sb = big.tile([NR, O], BF16, tag="bwT_sb")
            nc.sync.dma_start(bwT_sb, bwT)
            aT_sb = []
            for k in range(KT):
                a_t = big.tile([128, NR], BF16, tag=f"aT{k}")
                nc.sync.dma_start(a_t, aT[k * 128:(k + 1) * 128, :])
                aT_sb.append(a_t)
            xT_sb, wT_sb = [], []
            for k in range(KT):
                x_t = big.tile([128, T], BF16, tag=f"xT{k}")
                nc.sync.dma_start(x_t, xT[k * 128:(k + 1) * 128, :])
                xT_sb.append(x_t)
                w_t = big.tile([128, O], BF16, tag=f"wT{k}")
                nc.sync.dma_start(w_t, wT[k * 128:(k + 1) * 128, :])
                wT_sb.append(w_t)

            # ---- h phase: hT[(n,r), t], accumulated over k ----
            h_ps = [
                psum.tile([NR, 512], F32, tag="ps", name=f"h_ps{c}")
                for c in range(NCH)
            ]
            for k in range(KT):
                for c in range(NCH):
                    nc.tensor.matmul(
                        h_ps[c],
                        aT_sb[k],
                        xT_sb[k][:, c * 512:(c + 1) * 512],
                        start=(k == 0),
                        stop=(k == KT - 1),
                    )

            # ---- g = h * (mask * SCALE), cast to bf16 ----
            gT_sb = big.tile([NR, T], BF16, tag="gT_sb")
            for c in range(NCH):
                nc.vector.tensor_mul(
                    gT_sb[:, c * 512:(c + 1) * 512],
                    h_ps[c],
                    m64_sb[:, c * 512:(c + 1) * 512],
                )

            # ---- main phase: y[t,o] = x@W^T + g@BwT ----
            for t in range(NTS):
                ys = [
                    psum.tile([128, 512], F32, tag="ps", name=f"y_ps{t}_{o}")
                    for o in range(NO)
                ]
                ts = slice(t * 128, (t + 1) * 128)
                for k in range(KT):
                    lhsT = xT_sb[k][:, ts]
                    for o in range(NO):
                        nc.tensor.matmul(
                            ys[o],
                            lhsT,
                            wT_sb[k][:, o * 512:(o + 1) * 512],
                            start=(k == 0),
                            stop=False,
                        )
                gl = gT_sb[:, ts]
                for o in range(NO):
                    nc.tensor.matmul(
                        ys[o],
                        gl,
                        bwT_sb[:, o * 512:(o + 1) * 512],
                        start=False,
                        stop=True,
                    )
                for o in range(NO):
                    ot = outp.tile([128, 512], F32, tag="out", name=f"ot{t}_{o}")
                    nc.vector.tensor_copy(ot, ys[o])
                    nc.sync.dma_start(y[ts, o * 512:(o + 1) * 512], ot)

    nc.compile()
    return nc


def _get_nc():
    global _NC
    if _NC is None:
        _NC = _build()
    return _NC


def _install_ntff_shim():
    """Optional: register the axon NTFF profile hook so trace=True works."""
    import types
    import antenv
    if "antenv.axon_hooks" in sys.modules:
        return
    hook = [None]
    mod = types.ModuleType("antenv.axon_hooks")
    mod.set_axon_ntff_profile_hook = lambda h: hook.__setitem__(0, h)
    mod.get_axon_ntff_profile_hook = lambda: hook[0]
    sys.modules["antenv.axon_hooks"] = mod
    antenv.axon_hooks = mod
    from trn_agent_boot.trn_boot import _ntff_profile_via_ctypes
    mod.set_axon_ntff_profile_hook(
        _ntff_profile_via_ctypes("/opt/axon/libaxon_pjrt.so")
    )
    from concourse import bass_utils
    bass_utils.upload_artifacts = lambda tmpdir: tmpdir


def kernel(x, mask, W, b, A, Bw):
    x = np.asarray(x)
    mask = np.asarray(mask)
    W = np.asarray(W)
    b = np.asarray(b)
    A = np.asarray(A)
    Bw = np.asarray(Bw)

    B_, S, _ = x.shape
    bf16 = ml_dtypes.bfloat16

    xt = x.reshape(B_ * S, D).astype(bf16)               # [16384, D]
    WT = np.ascontiguousarray(W.astype(bf16).T)          # [D, O]
    AT = np.ascontiguousarray(A.reshape(NR, D).astype(bf16).T)          # [D, NR]
    BWT = np.ascontiguousarray(
        Bw.transpose(0, 2, 1).reshape(NR, O).astype(bf16)
    )                                                    # [NR, O]
    m2 = (mask.reshape(mask.shape[0], -1) * np.float32(SCALE)).astype(np.float32)
    m64_full = np.repeat(m2, NR // mask.shape[0], axis=0)  # [NR, 16384]

    in_maps = []
    for c in range(N_CORES):
        sl = slice(c * T, (c + 1) * T)
        in_maps.append({
            "xT": np.ascontiguousarray(xt[sl].T),
            "wT": WT,
            "aT": AT,
            "bwT": BWT,
            "m64": np.ascontiguousarray(m64_full[:, sl]),
        })

    nc = _get_nc()
    trace = os.environ.get("KERNEL_TRACE") == "1"
    if trace:
        try:
            _install_ntff_shim()
        except Exception as e:  # profiling is best-effort
            print(f"NTFF shim unavailable: {e}", file=sys.stderr)
            trace = False
    res = run_bass_kernel_spmd(
        nc, in_maps, core_ids=list(range(N_CORES)), trace=trace
    )
    kernel.last_exec_time_ns = res.exec_time_ns
    kernel.last_trace = res.instructions_and_trace

    yf = np.concatenate([res.results[c]["y"] for c in range(N_CORES)], axis=0)
    yf = yf + b.astype(np.float32)[None, :]
    return yf.reshape(B_, S, O).astype(np.float32)


# revision 5
# speedup vs baseline: 1.0496x; 1.0496x over previous
"""Trainium2 Bass kernel for masked multi-adaptor LoRA:

    y = x @ W^T + b + sum_n mask[n] * SCALE * ((x @ A[n]^T) @ Bw[n]^T)

Strategy (8 NeuronCores, data-parallel over tokens):
  - Flatten x to [B*S, D] = [16384, 2048] tokens; each core takes T=2048 tokens.
  - Host pre-transposes/casts to bf16: xT [D, T] per core, WT = W^T [D, O],
    packed AT [128, KT*64], BwT [(n,r), O], m64[(n,r), t] = mask[n,t]*SCALE.
  - Device per core:
      hT[(n,r), t] = AT_k.T @ xT_k summed over k   (PE, runs along the xT DMA stream)
      gT = hT * m64  (DVE, cast bf16)  -> stored as rows 0:64 of a [128, T] tile
      y[t, o] = sum_{k=0..16} xk[k].T @ wk[k]      (PE)
    where k=16 is the LoRA tail: xk[16] = gT17 (rows 64:128 zeroed), wk[16] =
    BwT padded with zero rows — a uniform K=128 matmul, so the whole main loop
    is 17 homogeneous k-steps accumulating in one PSUM bank per (t,o) tile.
  - First two token tiles run k-major (8 PSUM banks) so the PE follows the wT
    DMA stream; the rest run t-major from SBUF-resident data.
  - b is added on host (zeros in this problem, kept for generality).
"""

import os
import sys

if "/opt/trn_rl_repo" not in sys.path:
    sys.path.insert(0, "/opt/trn_rl_repo")

import numpy as np
import ml_dtypes

import concourse.mybir as mybir
import concourse.tile as tile
from concourse import bacc
from concourse.bass_utils import run_bass_kernel_spmd

N_CORES = 8
D = 2048          # d_in
O = 2048          # d_out
T = 2048          # tokens per core (16384 / 8)
NR = 64           # n_adaptors * r = 4 * 16
KT = D // 128     # 16 k-tiles
NCH = T // 512    # 512-token chunks for the h phase
NO = O // 512     # 512-wide output column tiles
NTS = T // 128    # 128-token output row tiles
SCALE = 2.0       # lora_alpha / r = 32 / 16
XG = 8            # xT dma groups (2 k-tiles each)
WG = 8            # wT dma groups

BF16 = mybir.dt.bfloat16
F32 = mybir.dt.float32

_NC = None


def _build():
    nc = bacc.Bacc("TRN2", target_bir_lowering=False, debug=False)
    xT = nc.dram_tensor("xT", [D, T], BF16, kind="ExternalInput").ap()
    wT = nc.dram_tensor("wT", [D, O], BF16, kind="ExternalInput").ap()
    aT = nc.dram_tensor("aT", [128, KT * NR], BF16, kind="ExternalInput").ap()
    bw17 = nc.dram_tensor("bw17", [NR, O], BF16, kind="ExternalInput").ap()
    m64 = nc.dram_tensor("m64", [NR, T], F32, kind="ExternalInput").ap()
    y = nc.dram_tensor("y", [T, O], F32, kind="ExternalOutput").ap()

    KX = KT // XG  # k-tiles per xT dma group
    KW = KT // WG

    with tile.TileContext(nc) as tc:
        with (
            tc.tile_pool(name="big", bufs=1) as big,
            tc.tile_pool(name="outp", bufs=3) as outp,
            tc.tile_pool(name="psum", bufs=8, space="PSUM") as psum,
        ):
            # ---- resident loads (trigger order matters: aT, xT..., m64, bw17, wT...) ----
            aT_sb = big.tile([128, KT * NR], BF16, tag="aT_sb")
            nc.sync.dma_start(aT_sb, aT)

            x_src = xT.rearrange("(g k p) t -> g p k t", g=XG, k=KX, p=128)
            xT_sb = []
            for g in range(XG):
                x_t = big.tile([128, KX * T], BF16, tag=f"xT{g}")
                nc.sync.dma_start(
                    x_t.rearrange("p (k t) -> p k t", k=KX), x_src[g]
                )
                xT_sb.append(x_t)

            m64_sb = big.tile([NR, T], F32, tag="m64_sb")
            nc.sync.dma_start(m64_sb, m64)

            wT17_sb = big.tile([128, O], BF16, tag="wT17_sb")
            nc.sync.dma_start(wT17_sb[0:NR, :], bw17)
            nc.gpsimd.memset(wT17_sb[NR:128, :], 0.0)

            gT17_sb = big.tile([128, T], BF16, tag="gT17_sb")
            nc.gpsimd.memset(gT17_sb[NR:128, :], 0.0)

            w_src = wT.rearrange("(g k p) o -> g p k o", g=WG, k=KW, p=128)
            wT_sb = []
            for g in range(WG):
                w_t = big.tile([128, KW * O], BF16, tag=f"wT{g}")
                nc.sync.dma_start(
                    w_t.rearrange("p (k o) -> p k o", k=KW), w_src[g]
                )
                wT_sb.append(w_t)

            # slice helpers: k in [0, 16] with 16 = LoRA tail
            def xk(k, c0, c1):
                if k == KT:
                    return gT17_sb[:, c0:c1]
                return xT_sb[k // KX][:, (k % KX) * T + c0:(k % KX) * T + c1]

            def wk(k, c0, c1):
                if k == KT:
                    return wT17_sb[:, c0:c1]
                return wT_sb[k // KW][:, (k % KW) * O + c0:(k % KW) * O + c1]

            # ---- h phase (k-major, follows the xT stream) ----
            h_ps = [
                psum.tile([NR, 512], F32, tag="ps", name=f"h_ps{c}")
                for c in range(NCH)
            ]
            for k in range(KT):
                a_sl = aT_sb[:, k * NR:(k + 1) * NR]
                for c in range(NCH):
                    nc.tensor.matmul(
                        h_ps[c],
                        a_sl,
                        xk(k, c * 512, (c + 1) * 512),
                        start=(k == 0),
                        stop=(k == KT - 1),
                    )

            # ---- g = h * (mask * SCALE) -> rows 0:64 of gT17 (bf16) ----
            for c in range(NCH):
                nc.vector.tensor_mul(
                    gT17_sb[0:NR, c * 512:(c + 1) * 512],
                    h_ps[c],
                    m64_sb[:, c * 512:(c + 1) * 512],
                )

            def drain(t, ys):
                ot = outp.tile([128, O], F32, tag="out", name=f"ot{t}")
                for o in range(NO):
                    nc.vector.tensor_copy(ot[:, o * 512:(o + 1) * 512], ys[o])
                nc.sync.dma_start(y[t * 128:(t + 1) * 128, :], ot)

            # ---- first two token tiles: k-major, follows the wT stream ----
            first = [
                [
                    psum.tile([128, 512], F32, tag="ps", name=f"y_ps{t}_{o}")
                    for o in range(NO)
                ]
                for t in range(2)
            ]
            for k in range(KT + 1):
                for t in range(2):
                    lhsT = xk(k, t * 128, (t + 1) * 128)
                    for o in range(NO):
                        nc.tensor.matmul(
                            first[t][o],
                            lhsT,
                            wk(k, o * 512, (o + 1) * 512),
                            start=(k == 0),
                            stop=(k == KT),
                        )
            for t in range(2):
                drain(t, first[t])

            # ---- remaining token tiles: t-major from resident SBUF ----
            for t in range(2, NTS):
                ys = [
                    psum.tile([128, 512], F32, tag="ps", name=f"y_ps{t}_{o}")
                    for o in range(NO)
                ]
                for k in range(KT + 1):
                    lhsT = xk(k, t * 128, (t + 1) * 128)
                    for o in range(NO):
                        nc.tensor.matmul(
                            ys[o],
                            lhsT,
                            wk(k, o * 512, (o + 1) * 512),
                            start=(k == 0),
                            stop=(k == KT),
                        )
                drain(t, ys)

    nc.compile()
    return nc


def _get_nc():
    global _NC
    if _NC is None:
        _NC = _build()
    return _NC


def _install_ntff_shim():
    """Optional: register the axon NTFF profile hook so trace=True works."""
    import types
    import antenv
    if "antenv.axon_hooks" in sys.modules:
        return
    hook = [None]
    mod = types.ModuleType("antenv.axon_hooks")
    mod.set_axon_ntff_profile_hook = lambda h: hook.__setitem__(0, h)
    mod.get_axon_ntff_profile_hook = lambda: hook[0]
    sys.modules["antenv.axon_hooks"] = mod
    antenv.axon_hooks = mod
    from trn_agent_boot.trn_boot import _ntff_profile_via_ctypes
    mod.set_axon_ntff_profile_hook(
        _ntff_profile_via_ctypes("/opt/axon/libaxon_pjrt.so")
    )
    from concourse import bass_utils
    bass_utils.upload_artifacts = lambda tmpdir: tmpdir


def kernel(x, mask, W, b, A, Bw):
    x = np.asarray(x)
    mask = np.asarray(mask)
    W = np.asarray(W)
    b = np.asarray(b)
    A = np.asarray(A)
    Bw = np.asarray(Bw)

    B_, S, _ = x.shape
    bf16 = ml_dtypes.bfloat16

    xt = x.reshape(B_ * S, D).astype(bf16)               # [16384, D]
    WT = np.ascontiguousarray(W.astype(bf16).T)          # [D, O]
    # packed A: aT[p, k*64+r] = A_cat[r, k*128+p]
    AT = np.ascontiguousarray(
        A.reshape(NR, KT, 128).transpose(2, 1, 0).reshape(128, KT * NR)
    ).astype(bf16)
    BWT = np.ascontiguousarray(
        Bw.transpose(0, 2, 1).reshape(NR, O).astype(bf16)
    )                                                    # [NR, O]
    m2 = (mask.reshape(mask.shape[0], -1) * np.float32(SCALE)).astype(np.float32)
    m64_full = np.repeat(m2, NR // mask.shape[0], axis=0)  # [NR, 16384]

    in_maps = []
    for c in range(N_CORES):
        sl = slice(c * T, (c + 1) * T)
        in_maps.append({
            "xT": np.ascontiguousarray(xt[sl].T),
            "wT": WT,
            "aT": AT,
            "bw17": BWT,
            "m64": np.ascontiguousarray(m64_full[:, sl]),
        })

    nc = _get_nc()
    trace = os.environ.get("KERNEL_TRACE") == "1"
    if trace:
        try:
            _install_ntff_shim()
        except Exception as e:  # profiling is best-effort
            print(f"NTFF shim unavailable: {e}", file=sys.stderr)
            trace = False
    res = run_bass_kernel_spmd(
        nc, in_maps, core_ids=list(range(N_CORES)), trace=trace
    )
    kernel.last_exec_time_ns = res.exec_time_ns
    kernel.last_trace = res.instructions_and_trace

    yf = np.concatenate([res.results[c]["y"] for c in range(N_CORES)], axis=0)
    yf = yf + b.astype(np.float32)[None, :]
    return yf.reshape(B_, S, O).astype(np.float32)


# revision 7
# speedup vs baseline: 1.0795x; 1.0285x over previous
"""Trainium2 Bass kernel for masked multi-adaptor LoRA:

    y = x @ W^T + b + sum_n mask[n] * SCALE * ((x @ A[n]^T) @ Bw[n]^T)

Strategy (8 NeuronCores, data-parallel over tokens):
  - Flatten x to [B*S, D] = [16384, 2048] tokens; each core takes T=2048 tokens.
  - Host pre-transposes/casts to bf16: xT [D, T] per core, WT = W^T [D, O],
    packed AT [128, KT*64], BwT [(n,r), O], m64[(n,r), t] = mask[n,t]*SCALE.
  - Device per core:
      hT[(n,r), t] = AT_k.T @ xT_k summed over k   (PE, runs along the xT DMA stream)
      gT = hT * m64  (DVE, cast bf16)  -> stored as rows 0:64 of a [128, T] tile
      y[t, o] = sum_{k=0..16} xk[k].T @ wk[k]      (PE)
    where k=16 is the LoRA tail: xk[16] = gT17 (rows 64:128 zeroed), wk[16] =
    BwT padded with zero rows — a uniform K=128 matmul, so the whole main loop
    is 17 homogeneous k-steps accumulating into PSUM.
  - First two token tiles run k-major (8 PSUM banks) so the PE follows the wT
    DMA stream; the rest run t-major from SBUF-resident data.
  - b is added on host (zeros in this problem, kept for generality).
"""

import os
import sys

if "/opt/trn_rl_repo" not in sys.path:
    sys.path.insert(0, "/opt/trn_rl_repo")

import numpy as np
import ml_dtypes

import concourse.mybir as mybir
import concourse.tile as tile
from concourse import bacc
from concourse.bass_utils import run_bass_kernel_spmd

N_CORES = 8
D = 2048          # d_in
O = 2048          # d_out
T = 2048          # tokens per core (16384 / 8)
NR = 64           # n_adaptors * r = 4 * 16
KT = D // 128     # 16 k-tiles
SCALE = 2.0       # lora_alpha / r = 32 / 16
XG = 8            # xT dma groups (2 k-tiles each)
WG = 8            # wT dma groups

FREE = 512        # moving-operand width (1024 fails: one matmul output <= one PSUM bank)
NOF = O // FREE   # output column tiles per token tile
NCH = T // FREE   # h-phase chunks
NTS = T // 128    # 128-token output row tiles

BF16 = mybir.dt.bfloat16
F32 = mybir.dt.float32

_NC = None


def _build():
    nc = bacc.Bacc("TRN2", target_bir_lowering=False, debug=False)
    xT = nc.dram_tensor("xT", [D, T], BF16, kind="ExternalInput").ap()
    wT = nc.dram_tensor("wT", [D, O], BF16, kind="ExternalInput").ap()
    aT = nc.dram_tensor("aT", [128, KT * NR], BF16, kind="ExternalInput").ap()
    bw17 = nc.dram_tensor("bw17", [NR, O], BF16, kind="ExternalInput").ap()
    m64 = nc.dram_tensor("m64", [NR, T], F32, kind="ExternalInput").ap()
    y = nc.dram_tensor("y", [T, O], F32, kind="ExternalOutput").ap()

    KX = KT // XG  # k-tiles per xT dma group
    KW = KT // WG

    with tile.TileContext(nc) as tc:
        with (
            tc.tile_pool(name="big", bufs=1) as big,
            tc.tile_pool(name="outp", bufs=3) as outp,
            tc.tile_pool(name="psum", bufs=8 * 512 // FREE, space="PSUM") as psum,
        ):
            # ---- resident loads; trigger order = arrival order ----
            aT_sb = big.tile([128, KT * NR], BF16, tag="aT_sb")
            nc.sync.dma_start(aT_sb, aT)

            m64_sb = big.tile([NR, T], F32, tag="m64_sb")
            nc.sync.dma_start(m64_sb, m64)

            wT17_sb = big.tile([128, O], BF16, tag="wT17_sb")
            nc.sync.dma_start(wT17_sb[0:NR, :], bw17)
            nc.gpsimd.memset(wT17_sb[NR:128, :], 0.0)

            gT17_sb = big.tile([128, T], BF16, tag="gT17_sb")
            nc.gpsimd.memset(gT17_sb[NR:128, :], 0.0)

            x_src = xT.rearrange("(g k p) t -> g p k t", g=XG, k=KX, p=128)
            xT_sb = []
            for g in range(XG):
                x_t = big.tile([128, KX * T], BF16, tag=f"xT{g}")
                nc.sync.dma_start(
                    x_t.rearrange("p (k t) -> p k t", k=KX), x_src[g]
                )
                xT_sb.append(x_t)

            w_src = wT.rearrange("(g k p) o -> g p k o", g=WG, k=KW, p=128)
            wT_sb = []
            for g in range(WG):
                w_t = big.tile([128, KW * O], BF16, tag=f"wT{g}")
                nc.sync.dma_start(
                    w_t.rearrange("p (k o) -> p k o", k=KW), w_src[g]
                )
                wT_sb.append(w_t)

            # slice helpers: k in [0, 16] with 16 = LoRA tail
            def xk(k, c0, c1):
                if k == KT:
                    return gT17_sb[:, c0:c1]
                return xT_sb[k // KX][:, (k % KX) * T + c0:(k % KX) * T + c1]

            def wk(k, c0, c1):
                if k == KT:
                    return wT17_sb[:, c0:c1]
                return wT_sb[k // KW][:, (k % KW) * O + c0:(k % KW) * O + c1]

            # ---- h phase (k-major, follows the xT stream) ----
            h_ps = [
                psum.tile([NR, FREE], F32, tag="ps", name=f"h_ps{c}")
                for c in range(NCH)
            ]
            for k in range(KT):
                a_sl = aT_sb[:, k * NR:(k + 1) * NR]
                for c in range(NCH):
                    nc.tensor.matmul(
                        h_ps[c],
                        a_sl,
                        xk(k, c * FREE, (c + 1) * FREE),
                        start=(k == 0),
                        stop=(k == KT - 1),
                    )

            # ---- g = h * (mask * SCALE) -> rows 0:64 of gT17 (bf16) ----
            for c in range(NCH):
                nc.vector.tensor_mul(
                    gT17_sb[0:NR, c * FREE:(c + 1) * FREE],
                    h_ps[c],
                    m64_sb[:, c * FREE:(c + 1) * FREE],
                )

            def drain(t, ys, split):
                ot = outp.tile([128, O], F32, tag="out", name=f"ot{t}")
                for o in range(NOF):
                    nc.vector.tensor_copy(ot[:, o * FREE:(o + 1) * FREE], ys[o])
                    if split:
                        nc.sync.dma_start(
                            y[t * 128:(t + 1) * 128, o * FREE:(o + 1) * FREE],
                            ot[:, o * FREE:(o + 1) * FREE],
                        )
                if not split:
                    nc.sync.dma_start(y[t * 128:(t + 1) * 128, :], ot)

            # ---- first two token tiles: k-major, follows the wT stream ----
            first = [
                [
                    psum.tile([128, FREE], F32, tag="ps", name=f"y_ps{t}_{o}")
                    for o in range(NOF)
                ]
                for t in range(2)
            ]
            for k in range(KT + 1):
                for t in range(2):
                    lhsT = xk(k, t * 128, (t + 1) * 128)
                    for o in range(NOF):
                        nc.tensor.matmul(
                            first[t][o],
                            lhsT,
                            wk(k, o * FREE, (o + 1) * FREE),
                            start=(k == 0),
                            stop=(k == KT),
                        )
            for t in range(2):
                drain(t, first[t], split=False)

            # ---- remaining token tiles: t-major from resident SBUF ----
            for t in range(2, NTS):
                ys = [
                    psum.tile([128, FREE], F32, tag="ps", name=f"y_ps{t}_{o}")
                    for o in range(NOF)
                ]
                for k in range(KT + 1):
                    lhsT = xk(k, t * 128, (t + 1) * 128)
                    for o in range(NOF):
                        nc.tensor.matmul(
                            ys[o],
                            lhsT,
                            wk(k, o * FREE, (o + 1) * FREE),
                            start=(k == 0),
                            stop=(k == KT),
                        )
                drain(t, ys, split=(t == NTS - 1))

    nc.compile()
    return nc


def _get_nc():
    global _NC
    if _NC is None:
        _NC = _build()
    return _NC


def _install_ntff_shim():
    """Optional: register the axon NTFF profile hook so trace=True works."""
    import types
    import antenv
    if "antenv.axon_hooks" in sys.modules:
        return
    hook = [None]
    mod = types.ModuleType("antenv.axon_hooks")
    mod.set_axon_ntff_profile_hook = lambda h: hook.__setitem__(0, h)
    mod.get_axon_ntff_profile_hook = lambda: hook[0]
    sys.modules["antenv.axon_hooks"] = mod
    antenv.axon_hooks = mod
    from trn_agent_boot.trn_boot import _ntff_profile_via_ctypes
    mod.set_axon_ntff_profile_hook(
        _ntff_profile_via_ctypes("/opt/axon/libaxon_pjrt.so")
    )
    from concourse import bass_utils
    bass_utils.upload_artifacts = lambda tmpdir: tmpdir


def kernel(x, mask, W, b, A, Bw):
    x = np.asarray(x)
    mask = np.asarray(mask)
    W = np.asarray(W)
    b = np.asarray(b)
    A = np.asarray(A)
    Bw = np.asarray(Bw)

    B_, S, _ = x.shape
    bf16 = ml_dtypes.bfloat16

    xt = x.reshape(B_ * S, D).astype(bf16)               # [16384, D]
    WT = np.ascontiguousarray(W.astype(bf16).T)          # [D, O]
    # packed A: aT[p, k*64+r] = A_cat[r, k*128+p]
    AT = np.ascontiguousarray(
        A.reshape(NR, KT, 128).transpose(2, 1, 0).reshape(128, KT * NR)
    ).astype(bf16)
    BWT = np.ascontiguousarray(
        Bw.transpose(0, 2, 1).reshape(NR, O).astype(bf16)
    )                                                    # [NR, O]
    m2 = (mask.reshape(mask.shape[0], -1) * np.float32(SCALE)).astype(np.float32)
    m64_full = np.repeat(m2, NR // mask.shape[0], axis=0)  # [NR, 16384]

    in_maps = []
    for c in range(N_CORES):
        sl = slice(c * T, (c + 1) * T)
        in_maps.append({
            "xT": np.ascontiguousarray(xt[sl].T),
            "wT": WT,
            "aT": AT,
            "bw17": BWT,
            "m64": np.ascontiguousarray(m64_full[:, sl]),
        })

    nc = _get_nc()
    trace = os.environ.get("KERNEL_TRACE") == "1"
    if trace:
        try:
            _install_ntff_shim()
        except Exception as e:  # profiling is best-effort
            print(f"NTFF shim unavailable: {e}", file=sys.stderr)
            trace = False
    res = run_bass_kernel_spmd(
        nc, in_maps, core_ids=list(range(N_CORES)), trace=trace
    )
    kernel.last_exec_time_ns = res.exec_time_ns
    kernel.last_trace = res.instructions_and_trace

    yf = np.concatenate([res.results[c]["y"] for c in range(N_CORES)], axis=0)
    yf = yf + b.astype(np.float32)[None, :]
    return yf.reshape(B_, S, O).astype(np.float32)
